# revision 10
# baseline (speedup 1.0000x reference)
"""Trainium2 Bass kernel for AttentionPooling (segment softmax pooling).

Math (reference):
    gate = x @ Wg + bg                 (N,)
    w    = segment_softmax(gate, index)
    out  = segment_sum(w * (x @ Wm + bm))          (S, D)

Algebraic refactor (exact up to fp rounding / the 1e-10 eps):
  - softmax max-subtraction dropped: gate ~ N(0,1), exp(gate) safe in fp32;
    bg cancels in the softmax.
  - pool first, then Wm:  out_s = (sum_r e_r x_r) @ Wm / (sum_r e_r) + bm.
  - x shipped PRE-SCALED: x' = x * wg  (same bytes). Then
    gate = rowsum(x'), and pooled' = sum_r e_r x'_r = pooled * wg, undone by
    folding diag(1/wg) into Wm on the host (relative bf16 errors are
    preserved under the per-feature scaling; psum accumulates in fp32).
    This turns the gate into a tensor_scalar+accum, which has a 4x DVE
    mode -- scalar_tensor_tensor (x*wg with accum) has NO fast modes.

Layout: rows sorted by segment id; each core takes an equal 125k-row chunk.
Rows packed into 128-row tiles, T tiles per block. Each TILE's rows span
< W segments (verified at prep): tile t scatters into psum columns
[t*W, (t+1)*W) -- compile-time offsets, identical on all cores (SPMD-safe).
Segments straddling tiles/cores produce partials the host scatter-adds.

Device per block (T tiles, bf16 on the wire; per-tile ops minimized --
GPSIMD/ACT have ~300-550ns fixed per-instruction overhead):
    per tile: gate_c = accum(tensor_scalar(x'_t))      DVE 4x   (~113ns)
    e = exp(gates)  [128,T] one op                     ACT
    eq = (iota_wt == bcast(idx))  [128,W,T] one op     GPSIMD
    ehot = eq * bcast(e)          [128,W,T] one op     DVE 2x
      ([W,T] layout keeps the bcast APs' last dim stride-1 -> fast modes ok)
    per tile: psum[:, tW:(t+1)W] += x'_t.T @ ehot[:,:,t]   PE (x stationary,
      output is pooled TRANSPOSED [feat, seg] -- no transpose stage)
    poolT -> SBUF (ACT Copy), Wm' matmul (PE), out -> SBUF (ACT Copy), DMA.
e ships once at the end; host computes esum (bincount), scatter-adds the
per-tile windows, normalizes, adds bm, zeroes empty segments.
"""
import sys
import numpy as np
import ml_dtypes

if "/opt/trn_rl_repo" not in sys.path:
    sys.path.insert(0, "/opt/trn_rl_repo")

BF16 = ml_dtypes.bfloat16

N, D, S, NC = 1_000_000, 128, 50_000, 8
RPC = N // NC                      # rows per core (125000)
NT = (RPC + 127) // 128            # real tiles per core (977)

# (tiles-per-block, per-tile segment window); first config whose window
# fits is used. T*W <= 512 (one PSUM bank).
CONFIGS = [(28, 16), (14, 32), (7, 64), (4, 128)]

# test-harness hooks
TRACE = False
LAST_RESULT = None


# ----------------------------------------------------------------- host prep
def _prep(x, Wg, index, T, W):
    """Sort rows by segment, pre-scale by wg, pack per-core blocks. Returns
    None if some 128-row tile spans >= W segments (caller tries next config)."""
    B = (NT + T - 1) // T
    GT = B * T                      # padded tiles per core
    RP = GT * 128                   # padded rows per core

    idx = np.ascontiguousarray(np.asarray(index)).astype(np.int64)
    x = np.ascontiguousarray(np.asarray(x), dtype=np.float32)
    wg = np.asarray(Wg, dtype=np.float32)[:, 0]
    order = np.argsort(idx, kind="stable")
    sidx = idx[order]               # [N] sorted segment ids

    sidx_pad = np.full((NC, RP), np.int64(1 << 40))
    sidx_pad[:, :RPC] = sidx.reshape(NC, RPC)
    tiles = sidx_pad.reshape(NC, GT, 128)
    win_base = tiles[:, :, 0].copy()             # [NC, GT] first-row seg
    loc = tiles - win_base[:, :, None]           # local idx within tile window
    real = tiles < (1 << 40)
    span = np.where(real, loc, 0).max()
    if span >= W:
        return None
    loc = np.where(real, loc, 300).astype(np.float32)

    # x' = x * wg, sorted, padded, bf16, packed [NC, B, 128, T*D]
    xs = (x[order] * wg[None, :]).astype(BF16)
    x_pad = np.zeros((NC, RP, D), dtype=BF16)
    x_pad[:, :RPC] = xs.reshape(NC, RPC, D)
    x_prep = np.ascontiguousarray(
        x_pad.reshape(NC, B, T, 128, D).transpose(0, 1, 3, 2, 4)
    ).reshape(NC, B, 128, T * D)

    # idx_all [NC, 128, GT]: column g = tile g's local ids (bf16-exact)
    idx_all = np.ascontiguousarray(
        loc.reshape(NC, GT, 128).transpose(0, 2, 1).astype(BF16))

    rows_in_tile = np.clip(RPC - np.arange(GT) * 128, 0, 128)
    return dict(x_prep=x_prep, idx_all=idx_all, win_base=win_base,
                sidx=sidx, B=B, T=T, W=W, rows_in_tile=rows_in_tile)


# --------------------------------------------------------------- bass program
def _build(B, T, W):
    import concourse.bacc as bacc
    import concourse.mybir as mybir
    from concourse.tile import TileContext

    dt = mybir.dt
    Alu = mybir.AluOpType
    Act = mybir.ActivationFunctionType
    GT = B * T

    nc = bacc.Bacc("TRN2", target_bir_lowering=False, debug=False, num_devices=NC)
    x_in = nc.dram_tensor("x_prep", [B, 128, T * D], dt.bfloat16,
                          kind="ExternalInput")
    idx_in = nc.dram_tensor("idx_all", [128, GT], dt.bfloat16,
                            kind="ExternalInput")
    iota_in = nc.dram_tensor("iota_wt", [128, W * T], dt.bfloat16,
                             kind="ExternalInput")
    wm_in = nc.dram_tensor("wm", [D, D], dt.bfloat16, kind="ExternalInput")
    out_st = nc.dram_tensor("out_stage", [B, 128, T * W], dt.bfloat16,
                            kind="ExternalOutput")
    e_st = nc.dram_tensor("e_stage", [128, GT], dt.bfloat16,
                          kind="ExternalOutput")

    with TileContext(nc) as tc:
        with tc.tile_pool(name="consts", bufs=1) as cpool, \
             tc.tile_pool(name="xblk", bufs=4) as xpool, \
             tc.tile_pool(name="junk", bufs=2) as jpool, \
             tc.tile_pool(name="ehot", bufs=4) as hpool, \
             tc.tile_pool(name="epi", bufs=4) as epool, \
             tc.tile_pool(name="psA", bufs=3, space="PSUM") as psA, \
             tc.tile_pool(name="psB", bufs=3, space="PSUM") as psB:

            wm_sb = cpool.tile([D, D], dt.bfloat16, tag="wm")
            nc.sync.dma_start(wm_sb[:], wm_in[:, :])
            idx_sb = cpool.tile([128, GT], dt.bfloat16, tag="idx")
            nc.sync.dma_start(idx_sb[:], idx_in[:, :])
            iota_sb = cpool.tile([128, W, T], dt.bfloat16, tag="iota")
            nc.sync.dma_start(iota_sb[:, :, :], iota_in[:, :])
            gate_sb = cpool.tile([128, GT], dt.float32, tag="gate")
            e_sb = cpool.tile([128, GT], dt.bfloat16, tag="e")

            # static one-hot skeleton for all blocks, built once upfront
            # (absorbed into the pipeline ramp; saves shipping it via DMA)
            eq_sb = cpool.tile([128, W, GT], dt.bfloat16, tag="eq")
            for b in range(B):
                idx_bc = idx_sb[:, b * T:(b + 1) * T].unsqueeze(1) \
                    .broadcast_to([128, W, T])
                nc.vector.tensor_tensor(
                    out=eq_sb[:, :, b * T:(b + 1) * T],
                    in0=iota_sb[:, :, :], in1=idx_bc, op=Alu.is_equal)

            # ACT gate tiles per block: ACT row-sums a few tiles via its
            # accumulator to offload the DVE reduce (both feed gate_sb);
            # alternate 3/4 to balance ACT vs DVE on average
            KA = 3

            def epilogue(b, ps):
                # delayed one block so ACT's in-order queue doesn't stall
                # the next block's exp behind these PSUM copies
                poolT = epool.tile([128, T * W], dt.bfloat16, tag="poolT_sb")
                nc.scalar.activation(poolT[:], ps[:], Act.Copy)
                psO = psB.tile([128, T * W], dt.float32, tag="psO")
                nc.tensor.matmul(psO[:], wm_sb[:], poolT[:],
                                 start=True, stop=True)
                out_sb = epool.tile([128, T * W], dt.bfloat16, tag="out")
                nc.scalar.activation(out_sb[:], psO[:], Act.Copy)
                nc.gpsimd.dma_start(out_st[b], out_sb[:])

            prev = None
            for b in range(B):
                ka = KA + (b & 1)
                xblk = xpool.tile([128, T, D], dt.bfloat16, tag="xblk")
                nc.sync.dma_start(xblk[:, :, :], x_in[b])

                # gate rowsums: one DVE reduce for tiles ka..T, ACT accum
                # for tiles 0..ka; then one batched exp (ACT)
                for t in range(ka):
                    c = b * T + t
                    junk = jpool.tile([128, D], dt.bfloat16, tag="junk")
                    nc.scalar.activation(junk[:], xblk[:, t, :], Act.Copy,
                                         accum_out=gate_sb[:, c:c + 1])
                nc.vector.tensor_reduce(
                    out=gate_sb[:, b * T + ka:(b + 1) * T],
                    in_=xblk[:, ka:, :],
                    axis=mybir.AxisListType.X, op=Alu.add)
                nc.scalar.activation(e_sb[:, b * T:(b + 1) * T],
                                     gate_sb[:, b * T:(b + 1) * T], Act.Exp)

                # ehot = eq * e on GPSIMD; [W, T] layout: T innermost so the
                # bcast/slice APs keep stride-1 tails
                e_bc = e_sb[:, b * T:(b + 1) * T].unsqueeze(1) \
                    .broadcast_to([128, W, T])
                ehot = hpool.tile([128, W, T], dt.bfloat16, tag="ehot")
                nc.gpsimd.tensor_tensor(
                    out=ehot[:, :, :],
                    in0=eq_sb[:, :, b * T:(b + 1) * T],
                    in1=e_bc, op=Alu.mult)

                # scatter: psum[:, t*W:(t+1)*W] += x'_t.T @ ehot[:, :, t]
                ps = psA.tile([128, T * W], dt.float32, tag="poolT")
                for t in range(T):
                    xt = xblk[:, t, :]
                    nc.tensor.matmul(ps[:, t * W:(t + 1) * W], xt,
                                     ehot[:, :, t],
                                     start=(t == 0), stop=(t == T - 1))

                if prev is not None:
                    epilogue(*prev)
                prev = (b, ps)
            epilogue(*prev)

            nc.sync.dma_start(e_st[:, :], e_sb[:])
    nc.compile()
    return nc


# -------------------------------------------------------------------- driver
def kernel(x, index, Wg, bg, Wm, bm, num_segments):
    from concourse.bass_utils import run_bass_kernel_spmd

    x = np.ascontiguousarray(np.asarray(x), dtype=np.float32)
    Wg = np.asarray(Wg, dtype=np.float32)
    Wm = np.asarray(Wm, dtype=np.float32)
    bm = np.asarray(bm, dtype=np.float32)

    layout = None
    for T, W in CONFIGS:
        layout = _prep(x, Wg, index, T, W)
        if layout is not None:
            break
    assert layout is not None, "tile segment window >128"
    B, T, W = layout["B"], layout["T"], layout["W"]
    GT = B * T

    nc = _build(B, T, W)

    # fold diag(1/wg) into Wm (undo the x' = x*wg pre-scaling after pooling)
    wg = Wg[:, 0].astype(np.float64)
    wg = np.where(np.abs(wg) < 1e-30, 1e-30, wg)
    wm_eff = (Wm.astype(np.float64) / wg[:, None]).astype(np.float32)
    wm_c = np.ascontiguousarray(wm_eff).astype(BF16)
    iota_wt = np.ascontiguousarray(np.broadcast_to(
        np.arange(W, dtype=np.float32)[None, :, None],
        (128, W, T))).reshape(128, W * T).astype(BF16)

    in_maps = []
    for c in range(NC):
        in_maps.append({
            "x_prep": layout["x_prep"][c],
            "idx_all": layout["idx_all"][c],
            "wm": wm_c,
            "iota_wt": iota_wt,
        })
    run_kwargs = {}
    if TRACE:
        run_kwargs = dict(trace=True, trace_cores=[0])
    res = run_bass_kernel_spmd(nc, in_maps, core_ids=list(range(NC)), **run_kwargs)
    global LAST_RESULT
    LAST_RESULT = res
    results = res.results

    sidx = layout["sidx"]
    rows_in_tile = layout["rows_in_tile"]
    win_base = layout["win_base"]
    acc = np.zeros((D, S + 128 + W), np.float32)   # [feat, seg] transposed
    esum = np.zeros(S, np.float64)
    for c in range(NC):
        outs = np.asarray(results[c]["out_stage"]).astype(np.float32)
        e_mat = np.asarray(results[c]["e_stage"]).astype(np.float32)
        e_rows = e_mat.T.reshape(-1)[:RPC]
        sidx_c = sidx[c * RPC:(c + 1) * RPC]
        esum += np.bincount(sidx_c, weights=e_rows.astype(np.float64),
                            minlength=S)
        for g in range(GT):
            if rows_in_tile[g] <= 0:
                continue
            wb = int(win_base[c, g])
            b, t = divmod(g, T)
            acc[:, wb:wb + W] += outs[b][:, t * W:(t + 1) * W]

    counts = np.bincount(np.asarray(index).astype(np.int64), minlength=S)
    esum_f = esum[:S].astype(np.float32)
    out = acc[:, :S].T / (esum_f[:, None] + np.float32(1e-10))
    out = out + bm[None, :]
    out[counts == 0] = 0.0
    return out.astype(np.float32)


# revision 12
# speedup vs baseline: 1.3116x; 1.3116x over previous
"""Trainium2 Bass kernel for AttentionPooling (segment softmax pooling).

Math (reference):
    gate = x @ Wg + bg                 (N,)
    w    = segment_softmax(gate, index)
    out  = segment_sum(w * (x @ Wm + bm))          (S, D)

Algebraic refactor (exact up to fp rounding / the 1e-10 eps):
  - softmax max-subtraction dropped: gate ~ N(0,1), exp(gate) safe in fp32;
    bg cancels in the softmax.
  - pool first, then Wm:  out_s = (sum_r e_r x_r) @ Wm / (sum_r e_r) + bm.
  - x shipped PRE-SCALED: x' = x * wg  (same bytes). Then
    gate = rowsum(x'), and pooled' = sum_r e_r x'_r = pooled * wg, undone by
    folding diag(1/wg) into Wm on the host (relative bf16 errors are
    preserved under the per-feature scaling; psum accumulates in fp32).
    This turns the gate into a tensor_scalar+accum, which has a 4x DVE
    mode -- scalar_tensor_tensor (x*wg with accum) has NO fast modes.

Layout: rows sorted by segment id; each core takes an equal 125k-row chunk.
Rows packed into 128-row tiles, T tiles per block. Each TILE's rows span
< W segments (verified at prep): tile t scatters into psum columns
[t*W, (t+1)*W) -- compile-time offsets, identical on all cores (SPMD-safe).
Segments straddling tiles/cores produce partials the host scatter-adds.

Device per block (T tiles, bf16 on the wire; per-tile ops minimized --
GPSIMD/ACT have ~300-550ns fixed per-instruction overhead):
    per tile: gate_c = accum(tensor_scalar(x'_t))      DVE 4x   (~113ns)
    e = exp(gates)  [128,T] one op                     ACT
    eq = (iota_wt == bcast(idx))  [128,W,T] one op     GPSIMD
    ehot = eq * bcast(e)          [128,W,T] one op     DVE 2x
      ([W,T] layout keeps the bcast APs' last dim stride-1 -> fast modes ok)
    per tile: psum[:, tW:(t+1)W] += x'_t.T @ ehot[:,:,t]   PE (x stationary,
      output is pooled TRANSPOSED [feat, seg] -- no transpose stage)
    poolT -> SBUF (ACT Copy), Wm' matmul (PE), out -> SBUF (ACT Copy), DMA.
e ships once at the end; host computes esum (bincount), scatter-adds the
per-tile windows, normalizes, adds bm, zeroes empty segments.
"""
import sys
import numpy as np
import ml_dtypes

if "/opt/trn_rl_repo" not in sys.path:
    sys.path.insert(0, "/opt/trn_rl_repo")

BF16 = ml_dtypes.bfloat16

N, D, S, NC = 1_000_000, 128, 50_000, 8
RPC = N // NC                      # rows per core (125000)
NT = (RPC + 127) // 128            # real tiles per core (977)

# (tiles-per-block, per-tile segment window); first config whose window
# fits is used. T*W <= 512 (one PSUM bank).
CONFIGS = [(28, 16), (14, 32), (7, 64), (4, 128)]

# test-harness hooks
TRACE = False
LAST_RESULT = None


# ----------------------------------------------------------------- host prep
def _prep(x, Wg, index, T, W):
    """Sort rows by segment, pre-scale by wg, pack per-core blocks. Returns
    None if some 128-row tile spans >= W segments (caller tries next config)."""
    B = (NT + T - 1) // T
    GT = B * T                      # padded tiles per core
    RP = GT * 128                   # padded rows per core

    idx = np.ascontiguousarray(np.asarray(index)).astype(np.int64)
    x = np.ascontiguousarray(np.asarray(x), dtype=np.float32)
    wg = np.asarray(Wg, dtype=np.float32)[:, 0]
    order = np.argsort(idx, kind="stable")
    sidx = idx[order]               # [N] sorted segment ids

    sidx_pad = np.full((NC, RP), np.int64(1 << 40))
    sidx_pad[:, :RPC] = sidx.reshape(NC, RPC)
    tiles = sidx_pad.reshape(NC, GT, 128)
    win_base = tiles[:, :, 0].copy()             # [NC, GT] first-row seg
    loc = tiles - win_base[:, :, None]           # local idx within tile window
    real = tiles < (1 << 40)
    span = np.where(real, loc, 0).max()
    if span >= W:
        return None
    loc = np.where(real, loc, 300).astype(np.float32)

    # x' = x * wg, sorted, padded, bf16, packed [NC, B, 128, T*D]
    xs = (x[order] * wg[None, :]).astype(BF16)
    x_pad = np.zeros((NC, RP, D), dtype=BF16)
    x_pad[:, :RPC] = xs.reshape(NC, RPC, D)
    x_prep = np.ascontiguousarray(
        x_pad.reshape(NC, B, T, 128, D).transpose(0, 1, 3, 2, 4)
    ).reshape(NC, B, 128, T * D)

    # idx_all [NC, 128, GT]: column g = tile g's local ids (bf16-exact)
    idx_all = np.ascontiguousarray(
        loc.reshape(NC, GT, 128).transpose(0, 2, 1).astype(BF16))

    rows_in_tile = np.clip(RPC - np.arange(GT) * 128, 0, 128)
    return dict(x_prep=x_prep, idx_all=idx_all, win_base=win_base,
                sidx=sidx, B=B, T=T, W=W, rows_in_tile=rows_in_tile)


# --------------------------------------------------------------- bass program
def _build(B, T, W):
    import concourse.bacc as bacc
    import concourse.mybir as mybir
    from concourse.tile import TileContext

    dt = mybir.dt
    Alu = mybir.AluOpType
    Act = mybir.ActivationFunctionType
    GT = B * T

    nc = bacc.Bacc("TRN2", target_bir_lowering=False, debug=False, num_devices=NC)
    x_in = nc.dram_tensor("x_prep", [B, 128, T * D], dt.bfloat16,
                          kind="ExternalInput")
    idx_in = nc.dram_tensor("idx_all", [128, GT], dt.bfloat16,
                            kind="ExternalInput")
    iota_in = nc.dram_tensor("iota_wt", [128, W * T], dt.bfloat16,
                             kind="ExternalInput")
    wm_in = nc.dram_tensor("wm", [D, D], dt.bfloat16, kind="ExternalInput")
    out_st = nc.dram_tensor("out_stage", [128, B * T * W], dt.bfloat16,
                            kind="ExternalOutput")
    e_st = nc.dram_tensor("e_stage", [128, GT], dt.bfloat16,
                          kind="ExternalOutput")

    with TileContext(nc) as tc:
        with tc.tile_pool(name="consts", bufs=1) as cpool, \
             tc.tile_pool(name="xblk", bufs=4) as xpool, \
             tc.tile_pool(name="junk", bufs=2) as jpool, \
             tc.tile_pool(name="ehot", bufs=4) as hpool, \
             tc.tile_pool(name="epi", bufs=4) as epool, \
             tc.tile_pool(name="psA", bufs=3, space="PSUM") as psA, \
             tc.tile_pool(name="psB", bufs=3, space="PSUM") as psB:

            wm_sb = cpool.tile([D, D], dt.bfloat16, tag="wm")
            nc.sync.dma_start(wm_sb[:], wm_in[:, :])
            idx_sb = cpool.tile([128, GT], dt.bfloat16, tag="idx")
            nc.sync.dma_start(idx_sb[:], idx_in[:, :])
            iota_sb = cpool.tile([128, W, T], dt.bfloat16, tag="iota")
            nc.sync.dma_start(iota_sb[:, :, :], iota_in[:, :])
            gate_sb = cpool.tile([128, GT], dt.float32, tag="gate")
            e_sb = cpool.tile([128, GT], dt.bfloat16, tag="e")

            out_all = cpool.tile([128, B * T * W], dt.bfloat16, tag="outall")

            # static one-hot skeleton for all blocks, built once upfront
            # (absorbed into the pipeline ramp; saves shipping it via DMA)
            eq_sb = cpool.tile([128, W, GT], dt.bfloat16, tag="eq")
            for b in range(B):
                idx_bc = idx_sb[:, b * T:(b + 1) * T].unsqueeze(1) \
                    .broadcast_to([128, W, T])
                nc.vector.tensor_tensor(
                    out=eq_sb[:, :, b * T:(b + 1) * T],
                    in0=iota_sb[:, :, :], in1=idx_bc, op=Alu.is_equal)

            # ACT gate tiles per block: ACT row-sums a few tiles via its
            # accumulator to offload the DVE reduce (both feed gate_sb);
            # alternate 3/4 to balance ACT vs DVE on average
            KA = 3

            CHUNK = 7    # blocks per out-DMA flush

            def epilogue(b, ps):
                # delayed one block so ACT's in-order queue doesn't stall
                # the next block's exp behind these PSUM copies
                poolT = epool.tile([128, T * W], dt.bfloat16, tag="poolT_sb")
                nc.scalar.activation(poolT[:], ps[:], Act.Copy)
                psO = psB.tile([128, T * W], dt.float32, tag="psO")
                nc.tensor.matmul(psO[:], wm_sb[:], poolT[:],
                                 start=True, stop=True)
                nc.scalar.activation(
                    out_all[:, b * T * W:(b + 1) * T * W], psO[:], Act.Copy)
                if b % CHUNK == CHUNK - 1 or b == B - 1:
                    lo = (b // CHUNK) * CHUNK
                    nc.scalar.dma_start(
                        out_st[:, lo * T * W:(b + 1) * T * W],
                        out_all[:, lo * T * W:(b + 1) * T * W])

            prev = None
            for b in range(B):
                ka = KA + (b & 1)
                xblk = xpool.tile([128, T, D], dt.bfloat16, tag="xblk")
                nc.sync.dma_start(xblk[:, :, :], x_in[b])

                # gate rowsums: one DVE reduce for tiles ka..T, ACT accum
                # for tiles 0..ka; then one batched exp (ACT)
                for t in range(ka):
                    c = b * T + t
                    junk = jpool.tile([128, D], dt.bfloat16, tag="junk")
                    nc.scalar.activation(junk[:], xblk[:, t, :], Act.Copy,
                                         accum_out=gate_sb[:, c:c + 1])
                nc.vector.tensor_reduce(
                    out=gate_sb[:, b * T + ka:(b + 1) * T],
                    in_=xblk[:, ka:, :],
                    axis=mybir.AxisListType.X, op=Alu.add)
                nc.scalar.activation(e_sb[:, b * T:(b + 1) * T],
                                     gate_sb[:, b * T:(b + 1) * T], Act.Exp)

                # ehot = eq * e on GPSIMD; [W, T] layout: T innermost so the
                # bcast/slice APs keep stride-1 tails
                e_bc = e_sb[:, b * T:(b + 1) * T].unsqueeze(1) \
                    .broadcast_to([128, W, T])
                ehot = hpool.tile([128, W, T], dt.bfloat16, tag="ehot")
                nc.gpsimd.tensor_tensor(
                    out=ehot[:, :, :],
                    in0=eq_sb[:, :, b * T:(b + 1) * T],
                    in1=e_bc, op=Alu.mult)

                # scatter: psum[:, t*W:(t+1)*W] += x'_t.T @ ehot[:, :, t]
                ps = psA.tile([128, T * W], dt.float32, tag="poolT")
                for t in range(T):
                    xt = xblk[:, t, :]
                    nc.tensor.matmul(ps[:, t * W:(t + 1) * W], xt,
                                     ehot[:, :, t],
                                     start=(t == 0), stop=(t == T - 1))

                if prev is not None:
                    epilogue(*prev)
                prev = (b, ps)
            epilogue(*prev)

            nc.sync.dma_start(e_st[:, :], e_sb[:])
    nc.compile()
    return nc


# -------------------------------------------------------------------- driver
def kernel(x, index, Wg, bg, Wm, bm, num_segments):
    from concourse.bass_utils import run_bass_kernel_spmd

    x = np.ascontiguousarray(np.asarray(x), dtype=np.float32)
    Wg = np.asarray(Wg, dtype=np.float32)
    Wm = np.asarray(Wm, dtype=np.float32)
    bm = np.asarray(bm, dtype=np.float32)

    layout = None
    for T, W in CONFIGS:
        layout = _prep(x, Wg, index, T, W)
        if layout is not None:
            break
    assert layout is not None, "tile segment window >128"
    B, T, W = layout["B"], layout["T"], layout["W"]
    GT = B * T

    nc = _build(B, T, W)

    # fold diag(1/wg) into Wm (undo the x' = x*wg pre-scaling after pooling)
    wg = Wg[:, 0].astype(np.float64)
    wg = np.where(np.abs(wg) < 1e-30, 1e-30, wg)
    wm_eff = (Wm.astype(np.float64) / wg[:, None]).astype(np.float32)
    wm_c = np.ascontiguousarray(wm_eff).astype(BF16)
    iota_wt = np.ascontiguousarray(np.broadcast_to(
        np.arange(W, dtype=np.float32)[None, :, None],
        (128, W, T))).reshape(128, W * T).astype(BF16)

    in_maps = []
    for c in range(NC):
        in_maps.append({
            "x_prep": layout["x_prep"][c],
            "idx_all": layout["idx_all"][c],
            "wm": wm_c,
            "iota_wt": iota_wt,
        })
    run_kwargs = {}
    if TRACE:
        run_kwargs = dict(trace=True, trace_cores=[0])
    res = run_bass_kernel_spmd(nc, in_maps, core_ids=list(range(NC)), **run_kwargs)
    global LAST_RESULT
    LAST_RESULT = res
    results = res.results

    sidx = layout["sidx"]
    rows_in_tile = layout["rows_in_tile"]
    win_base = layout["win_base"]
    acc = np.zeros((D, S + 128 + W), np.float32)   # [feat, seg] transposed
    esum = np.zeros(S, np.float64)
    for c in range(NC):
        outs = np.asarray(results[c]["out_stage"]).astype(np.float32) \
            .reshape(128, B, T, W)
        e_mat = np.asarray(results[c]["e_stage"]).astype(np.float32)
        e_rows = e_mat.T.reshape(-1)[:RPC]
        sidx_c = sidx[c * RPC:(c + 1) * RPC]
        esum += np.bincount(sidx_c, weights=e_rows.astype(np.float64),
                            minlength=S)
        for g in range(GT):
            if rows_in_tile[g] <= 0:
                continue
            wb = int(win_base[c, g])
            b, t = divmod(g, T)
            acc[:, wb:wb + W] += outs[:, b, t, :]

    counts = np.bincount(np.asarray(index).astype(np.int64), minlength=S)
    esum_f = esum[:S].astype(np.float32)
    out = acc[:, :S].T / (esum_f[:, None] + np.float32(1e-10))
    out = out + bm[None, :]
    out[counts == 0] = 0.0
    return out.astype(np.float32)


# revision 13
# speedup vs baseline: 1.5960x; 1.2169x over previous
"""Trainium2 Bass kernel for AttentionPooling (segment softmax pooling).

Math (reference):
    gate = x @ Wg + bg                 (N,)
    w    = segment_softmax(gate, index)
    out  = segment_sum(w * (x @ Wm + bm))          (S, D)

Algebraic refactor (exact up to fp rounding / the 1e-10 eps):
  - softmax max-subtraction dropped: gate ~ N(0,1), exp(gate) safe in fp32;
    bg cancels in the softmax.
  - pool first, then Wm:  out_s = (sum_r e_r x_r) @ Wm / (sum_r e_r) + bm.
  - x shipped PRE-SCALED: x' = x * wg  (same bytes). Then
    gate = rowsum(x'), and pooled' = sum_r e_r x'_r = pooled * wg, undone by
    folding diag(1/wg) into Wm on the host (relative bf16 errors are
    preserved under the per-feature scaling; psum accumulates in fp32).
    This turns the gate into a tensor_scalar+accum, which has a 4x DVE
    mode -- scalar_tensor_tensor (x*wg with accum) has NO fast modes.

Layout: rows sorted by segment id; each core takes an equal 125k-row chunk.
Rows packed into 128-row tiles, T tiles per block. Each TILE's rows span
< W segments (verified at prep): tile t scatters into psum columns
[t*W, (t+1)*W) -- compile-time offsets, identical on all cores (SPMD-safe).
Segments straddling tiles/cores produce partials the host scatter-adds.

Device per block (T tiles, bf16 on the wire; per-tile smalls minimized --
GPSIMD/ACT have ~300-550ns fixed per-instruction overhead):
    gates: ONE DVE tensor_reduce over [128, T-KA, D] + KA tiles row-summed
      on ACT via activation(Copy, accum_out) to balance the two engines
    e = exp(gates)  [128,T] one op                     ACT
    ehot = eq * bcast(e)  [128,W,T] one op             GPSIMD Multiply
      (eq = one-hot skeleton, built once upfront on DVE from idx/iota;
       [W,T] layout keeps slice/bcast APs' last dim stride-1)
    per tile: psum[:, tW:(t+1)W] += x'_t.T @ ehot[:,:,t]   PE (x stationary,
      output is pooled TRANSPOSED [feat, seg] -- no transpose stage)
    poolT -> SBUF (ACT Copy), Wm' matmul (PE), out -> SBUF (ACT Copy), DMA;
    the epilogue is emitted one block late so ACT's in-order queue never
    stalls the next block's exp behind PSUM copies.
e ships once at the end; host computes esum (bincount), scatter-adds the
per-tile windows, normalizes, adds bm, zeroes empty segments.
"""
import sys
import numpy as np
import ml_dtypes

if "/opt/trn_rl_repo" not in sys.path:
    sys.path.insert(0, "/opt/trn_rl_repo")

BF16 = ml_dtypes.bfloat16

N, D, S, NC = 1_000_000, 128, 50_000, 8
RPC = N // NC                      # rows per core (125000)
NT = (RPC + 127) // 128            # real tiles per core (977)

# (tiles-per-block, per-tile segment window); first config whose window
# fits is used. T*W <= 512 (one PSUM bank).
CONFIGS = [(28, 16), (14, 32), (7, 64), (4, 128)]

# test-harness hooks
TRACE = False
LAST_RESULT = None


# ----------------------------------------------------------------- host prep
def _prep(x, Wg, index, T, W):
    """Sort rows by segment, pre-scale by wg, pack per-core blocks. Returns
    None if some 128-row tile spans >= W segments (caller tries next config)."""
    B = (NT + T - 1) // T
    GT = B * T                      # padded tiles per core
    RP = GT * 128                   # padded rows per core

    idx = np.ascontiguousarray(np.asarray(index)).astype(np.int64)
    x = np.ascontiguousarray(np.asarray(x), dtype=np.float32)
    wg = np.asarray(Wg, dtype=np.float32)[:, 0]
    order = np.argsort(idx, kind="stable")
    sidx = idx[order]               # [N] sorted segment ids

    sidx_pad = np.full((NC, RP), np.int64(1 << 40))
    sidx_pad[:, :RPC] = sidx.reshape(NC, RPC)
    tiles = sidx_pad.reshape(NC, GT, 128)
    win_base = tiles[:, :, 0].copy()             # [NC, GT] first-row seg
    loc = tiles - win_base[:, :, None]           # local idx within tile window
    real = tiles < (1 << 40)
    span = np.where(real, loc, 0).max()
    if span >= W:
        return None
    loc = np.where(real, loc, 300).astype(np.float32)

    # x' = x * wg, sorted, padded, bf16, packed [NC, B, 128, T*D]
    xs = (x[order] * wg[None, :]).astype(BF16)
    x_pad = np.zeros((NC, RP, D), dtype=BF16)
    x_pad[:, :RPC] = xs.reshape(NC, RPC, D)
    x_prep = np.ascontiguousarray(
        x_pad.reshape(NC, B, T, 128, D).transpose(0, 1, 3, 2, 4)
    ).reshape(NC, B, 128, T * D)

    # idx_all [NC, 128, GT]: column g = tile g's local ids (bf16-exact)
    idx_all = np.ascontiguousarray(
        loc.reshape(NC, GT, 128).transpose(0, 2, 1).astype(BF16))

    rows_in_tile = np.clip(RPC - np.arange(GT) * 128, 0, 128)
    return dict(x_prep=x_prep, idx_all=idx_all, win_base=win_base,
                sidx=sidx, B=B, T=T, W=W, rows_in_tile=rows_in_tile)


# --------------------------------------------------------------- bass program
def _build(B, T, W):
    import concourse.bacc as bacc
    import concourse.mybir as mybir
    from concourse.tile import TileContext

    dt = mybir.dt
    Alu = mybir.AluOpType
    Act = mybir.ActivationFunctionType
    GT = B * T

    nc = bacc.Bacc("TRN2", target_bir_lowering=False, debug=False, num_devices=NC)
    x_in = nc.dram_tensor("x_prep", [B, 128, T * D], dt.bfloat16,
                          kind="ExternalInput")
    idx_in = nc.dram_tensor("idx_all", [128, GT], dt.bfloat16,
                            kind="ExternalInput")
    iota_in = nc.dram_tensor("iota_wt", [128, W * T], dt.bfloat16,
                             kind="ExternalInput")
    wm_in = nc.dram_tensor("wm", [D, D], dt.bfloat16, kind="ExternalInput")
    out_st = nc.dram_tensor("out_stage", [B, 128, T * W], dt.bfloat16,
                            kind="ExternalOutput")
    e_st = nc.dram_tensor("e_stage", [128, GT], dt.bfloat16,
                          kind="ExternalOutput")

    with TileContext(nc) as tc:
        with tc.tile_pool(name="consts", bufs=1) as cpool, \
             tc.tile_pool(name="xblk", bufs=4) as xpool, \
             tc.tile_pool(name="junk", bufs=2) as jpool, \
             tc.tile_pool(name="ehot", bufs=4) as hpool, \
             tc.tile_pool(name="epi", bufs=4) as epool, \
             tc.tile_pool(name="psA", bufs=3, space="PSUM") as psA, \
             tc.tile_pool(name="psB", bufs=3, space="PSUM") as psB:

            wm_sb = cpool.tile([D, D], dt.bfloat16, tag="wm")
            nc.sync.dma_start(wm_sb[:], wm_in[:, :])
            idx_sb = cpool.tile([128, GT], dt.bfloat16, tag="idx")
            nc.sync.dma_start(idx_sb[:], idx_in[:, :])
            iota_sb = cpool.tile([128, W, T], dt.bfloat16, tag="iota")
            nc.sync.dma_start(iota_sb[:, :, :], iota_in[:, :])
            gate_sb = cpool.tile([128, GT], dt.float32, tag="gate")
            e_sb = cpool.tile([128, GT], dt.bfloat16, tag="e")

            # static one-hot skeleton for all blocks, built once upfront
            # (absorbed into the pipeline ramp; saves shipping it via DMA)
            eq_sb = cpool.tile([128, W, GT], dt.bfloat16, tag="eq")
            for b in range(B):
                idx_bc = idx_sb[:, b * T:(b + 1) * T].unsqueeze(1) \
                    .broadcast_to([128, W, T])
                nc.vector.tensor_tensor(
                    out=eq_sb[:, :, b * T:(b + 1) * T],
                    in0=iota_sb[:, :, :], in1=idx_bc, op=Alu.is_equal)

            # ACT gate tiles per block: ACT row-sums a few tiles via its
            # accumulator to offload the DVE reduce (both feed gate_sb)
            KA = 4

            def epilogue(b, ps):
                # delayed one block so ACT's in-order queue doesn't stall
                # the next block's exp behind these PSUM copies
                poolT = epool.tile([128, T * W], dt.bfloat16, tag="poolT_sb")
                nc.scalar.activation(poolT[:], ps[:], Act.Copy)
                psO = psB.tile([128, T * W], dt.float32, tag="psO")
                nc.tensor.matmul(psO[:], wm_sb[:], poolT[:],
                                 start=True, stop=True)
                out_sb = epool.tile([128, T * W], dt.bfloat16, tag="out")
                nc.scalar.activation(out_sb[:], psO[:], Act.Copy)
                nc.scalar.dma_start(out_st[b], out_sb[:])

            prev = None
            for b in range(B):
                ka = KA
                xblk = xpool.tile([128, T, D], dt.bfloat16, tag="xblk")
                nc.sync.dma_start(xblk[:, :, :], x_in[b])

                # gate rowsums: one DVE reduce for tiles ka..T, ACT accum
                # for tiles 0..ka; then one batched exp (ACT)
                for t in range(ka):
                    c = b * T + t
                    junk = jpool.tile([128, D], dt.bfloat16, tag="junk")
                    nc.scalar.activation(junk[:], xblk[:, t, :], Act.Copy,
                                         accum_out=gate_sb[:, c:c + 1])
                nc.vector.tensor_reduce(
                    out=gate_sb[:, b * T + ka:(b + 1) * T],
                    in_=xblk[:, ka:, :],
                    axis=mybir.AxisListType.X, op=Alu.add)
                nc.scalar.activation(e_sb[:, b * T:(b + 1) * T],
                                     gate_sb[:, b * T:(b + 1) * T], Act.Exp)

                # ehot = eq * e on GPSIMD; [W, T] layout: T innermost so the
                # bcast/slice APs keep stride-1 tails
                e_bc = e_sb[:, b * T:(b + 1) * T].unsqueeze(1) \
                    .broadcast_to([128, W, T])
                ehot = hpool.tile([128, W, T], dt.bfloat16, tag="ehot")
                nc.gpsimd.tensor_tensor(
                    out=ehot[:, :, :],
                    in0=eq_sb[:, :, b * T:(b + 1) * T],
                    in1=e_bc, op=Alu.mult)

                # scatter: psum[:, t*W:(t+1)*W] += x'_t.T @ ehot[:, :, t]
                ps = psA.tile([128, T * W], dt.float32, tag="poolT")
                for t in range(T):
                    xt = xblk[:, t, :]
                    nc.tensor.matmul(ps[:, t * W:(t + 1) * W], xt,
                                     ehot[:, :, t],
                                     start=(t == 0), stop=(t == T - 1))

                if prev is not None:
                    epilogue(*prev)
                prev = (b, ps)
            epilogue(*prev)

            nc.sync.dma_start(e_st[:, :], e_sb[:])
    nc.compile()
    return nc


# -------------------------------------------------------------------- driver
def kernel(x, index, Wg, bg, Wm, bm, num_segments):
    from concourse.bass_utils import run_bass_kernel_spmd

    x = np.ascontiguousarray(np.asarray(x), dtype=np.float32)
    Wg = np.asarray(Wg, dtype=np.float32)
    Wm = np.asarray(Wm, dtype=np.float32)
    bm = np.asarray(bm, dtype=np.float32)

    layout = None
    for T, W in CONFIGS:
        layout = _prep(x, Wg, index, T, W)
        if layout is not None:
            break
    assert layout is not None, "tile segment window >128"
    B, T, W = layout["B"], layout["T"], layout["W"]
    GT = B * T

    nc = _build(B, T, W)

    # fold diag(1/wg) into Wm (undo the x' = x*wg pre-scaling after pooling)
    wg = Wg[:, 0].astype(np.float64)
    wg = np.where(np.abs(wg) < 1e-30, 1e-30, wg)
    wm_eff = (Wm.astype(np.float64) / wg[:, None]).astype(np.float32)
    wm_c = np.ascontiguousarray(wm_eff).astype(BF16)
    iota_wt = np.ascontiguousarray(np.broadcast_to(
        np.arange(W, dtype=np.float32)[None, :, None],
        (128, W, T))).reshape(128, W * T).astype(BF16)

    in_maps = []
    for c in range(NC):
        in_maps.append({
            "x_prep": layout["x_prep"][c],
            "idx_all": layout["idx_all"][c],
            "wm": wm_c,
            "iota_wt": iota_wt,
        })
    run_kwargs = {}
    if TRACE:
        run_kwargs = dict(trace=True, trace_cores=[0])
    res = run_bass_kernel_spmd(nc, in_maps, core_ids=list(range(NC)), **run_kwargs)
    global LAST_RESULT
    LAST_RESULT = res
    results = res.results

    sidx = layout["sidx"]
    rows_in_tile = layout["rows_in_tile"]
    win_base = layout["win_base"]
    acc = np.zeros((D, S + 128 + W), np.float32)   # [feat, seg] transposed
    esum = np.zeros(S, np.float64)
    for c in range(NC):
        outs = np.asarray(results[c]["out_stage"]).astype(np.float32)
        e_mat = np.asarray(results[c]["e_stage"]).astype(np.float32)
        e_rows = e_mat.T.reshape(-1)[:RPC]
        sidx_c = sidx[c * RPC:(c + 1) * RPC]
        esum += np.bincount(sidx_c, weights=e_rows.astype(np.float64),
                            minlength=S)
        for g in range(GT):
            if rows_in_tile[g] <= 0:
                continue
            wb = int(win_base[c, g])
            b, t = divmod(g, T)
            acc[:, wb:wb + W] += outs[b][:, t * W:(t + 1) * W]

    counts = np.bincount(np.asarray(index).astype(np.int64), minlength=S)
    esum_f = esum[:S].astype(np.float32)
    out = acc[:, :S].T / (esum_f[:, None] + np.float32(1e-10))
    out = out + bm[None, :]
    out[counts == 0] = 0.0
    return out.astype(np.float32)
